# revision 2
# baseline (speedup 1.0000x reference)
"""AnisotropySuppressionLoss on 8 TRN2 NeuronCores (Bass/Tile).

Per image (1024x1024, fp32): 2D FFT via f32r matmuls using the real-input
half-spectrum (u = 0..512 with row weights), power spectrum, radial
segment sums over 725 integer-radius bins via a barrel-shift shear
(exact), and the loss identity
    loss_img = sum_w P^2 - sum_k S_k^2/c_k (+ HW*eps^2, added on host).
Data-parallel: batch 16 -> 2 images on each of 8 cores; host averages.
"""

import os
import sys

sys.path.insert(0, "/opt/trn_rl_repo")

import numpy as np

import concourse.bass as bass
import concourse.tile as tile
from concourse import bacc, mybir
from concourse.bass_utils import run_bass_kernel_spmd
from concourse.masks import make_identity

F32 = mybir.dt.float32
F32R = mybir.dt.float32r
BF16 = mybir.dt.bfloat16

H = W_IMG = 1024
NQ = 513          # quadrant size (|du|, |dv| in 0..512)
NB = 725          # radial bins 0..724
WB = 728          # barrel buffer width
NROUNDS = 8
N_CORES = 8
IMGS_PER_CORE = 2
WA = 0.002
EPS = 1e-12
CHUNKS = [(0, 128), (128, 256), (256, 384), (384, 512), (512, 513)]

_CACHE = {}


# ---------------------------------------------------------------- host consts
def _gen_barrel_masks():
    """Per (chunk, round): (lo, hi, move_mask[128, hi-lo]) in quadrant coords.
    Cells (a, b>=a) carry delta = bin - b; each round moves cells with bit t
    set right by 2^t. Merges are exact (same remaining delta); validated."""
    rem = -np.ones((NQ, WB), dtype=np.int64)
    for a in range(NQ):
        cols = np.arange(a, NQ)
        bins = np.floor(np.sqrt(a * a + cols.astype(np.float64) ** 2)).astype(np.int64)
        rem[a, cols] = bins - cols
    table = [[] for _ in CHUNKS]
    for t in range(NROUNDS):
        bit = 1 << t
        move = (rem >= 0) & ((rem & bit) != 0)
        for ci, (c0, c1) in enumerate(CHUNKS):
            mv = move[c0:c1]
            cols_any = np.nonzero(mv.any(axis=0))[0]
            if len(cols_any) == 0:
                table[ci].append((0, 0, None))
            else:
                lo, hi = int(cols_any[0]), int(cols_any[-1]) + 1
                m = np.zeros((128, hi - lo), dtype=np.float32)
                m[: c1 - c0] = mv[:, lo:hi]
                table[ci].append((lo, hi, m))
        new_rem = -np.ones_like(rem)
        stay = (rem >= 0) & ~move
        new_rem[stay] = rem[stay]
        sr, sc = np.nonzero(move)
        dc = sc + bit
        landing = rem[sr, sc] - bit
        cur = new_rem[sr, dc]
        assert ((cur == -1) | (cur == landing)).all()
        new_rem[sr, dc] = landing
        rem = new_rem
    assert (rem[rem >= 0] == 0).all()
    return table


def _host_constants():
    if "consts" in _CACHE:
        return _CACHE["consts"]
    r = np.arange(H, dtype=np.float64)
    ang = 2.0 * np.pi * np.outer(r, r) / H
    Cm = np.cos(ang).astype(np.float32)
    Sm = np.sin(ang).astype(np.float32)

    # radial bins exactly as reference._radial_bins (unshifted coords)
    y = np.minimum(np.arange(H), H - np.arange(H))  # |du| per row, 0..512
    yy, xx = np.meshgrid(y, y, indexing="ij")
    dist = np.sqrt((xx.astype(np.float64)) ** 2 + yy.astype(np.float64) ** 2)
    bins_full = np.clip(dist.astype(np.int32), 0, NB - 1)
    counts = np.bincount(bins_full.reshape(-1), minlength=NB).astype(np.float64)
    invc = np.zeros((1, WB), dtype=np.float32)
    invc[0, :NB] = (1.0 / counts).astype(np.float32)

    # row weights w_u for u = 0..512
    w = np.full(NQ, 2.0)
    w[0] = 1.0
    w[512] = 1.0
    swc = np.zeros((128, 10), dtype=np.float32)
    for mu in range(5):
        c0, c1 = CHUNKS[mu]
        n = c1 - c0
        swc[:n, 2 * mu] = (np.sqrt(w[c0:c1]) / H).astype(np.float32)
        swc[:n, 2 * mu + 1] = (1.0 / w[c0:c1]).astype(np.float32)

    table = _gen_barrel_masks()
    widths = [[(hi - lo) for (lo, hi, m) in table[ci]] for ci in range(5)]
    chunk_w = [max(1, sum(ws)) for ws in widths]
    maxw = max(chunk_w)
    bmask = np.zeros((640, maxw), dtype=np.float32)  # cast to bf16 at pack time
    for ci in range(5):
        off = 0
        for (lo, hi, m) in table[ci]:
            if m is None:
                continue
            bmask[128 * ci : 128 * ci + 128, off : off + hi - lo] = m
            off += hi - lo

    # full-width pre-masks: 0 below diag, 0.5 on diag (doubled by fold),
    # 1 above; chunk 4 keeps its single diagonal cell at weight 1 (no
    # transpose-add was applied to it).
    mfull = np.zeros((640, NQ), dtype=np.float32)
    for ci in range(4):
        a = 128 * ci + np.arange(128)
        cols = np.arange(NQ)
        blk = (cols[None, :] > a[:, None]).astype(np.float32)
        blk[np.arange(128), a] = 0.5
        mfull[128 * ci : 128 * ci + 128] = blk
    mfull[512, 512] = 1.0

    import ml_dtypes
    _CACHE["consts"] = dict(
        Cm=Cm, Sm=Sm, invc=invc, swc=swc, bmask=bmask,
        bmask_bf16=bmask.astype(ml_dtypes.bfloat16), mfull=mfull,
        mfull_bf16=mfull.astype(ml_dtypes.bfloat16),
        table=table, maxw=maxw, counts=counts,
    )
    return _CACHE["consts"]


# ---------------------------------------------------------------- device build
def _build_nc():
    hc = _host_constants()
    table, maxw = hc["table"], hc["maxw"]

    nc = bacc.Bacc("TRN2", target_bir_lowering=False, debug=False)
    x_p = nc.declare_dram_parameter("x", [IMGS_PER_CORE, H, H], F32R, isOutput=False)
    cm_p = nc.declare_dram_parameter("cm", [H, H], F32R, isOutput=False)
    sm_p = nc.declare_dram_parameter("sm", [H, H], F32R, isOutput=False)
    bm_p = nc.declare_dram_parameter("bm", [640, maxw], BF16, isOutput=False)
    mf_p = nc.declare_dram_parameter("mf", [640, NQ], BF16, isOutput=False)
    sw_p = nc.declare_dram_parameter("sw", [128, 10], F32, isOutput=False)
    ic_p = nc.declare_dram_parameter("ic", [1, WB], F32, isOutput=False)
    out_p = nc.declare_dram_parameter("out", [1, IMGS_PER_CORE], F32, isOutput=True)

    AT = mybir.AluOpType

    with tile.TileContext(nc) as tc:
        with (
            tc.tile_pool(name="const", bufs=1) as cpool,
            tc.tile_pool(name="xin", bufs=1) as xpool,
            tc.tile_pool(name="arr", bufs=1) as apool,
            tc.tile_pool(name="quad", bufs=1) as qpool,
            tc.tile_pool(name="work", bufs=2) as wpool,
            tc.tile_pool(name="ps", bufs=2, space="PSUM") as ps,
        ):
            # constants
            Cm_t = [cpool.tile([128, H], F32R, tag=f"cm{k}", name=f"cm{k}") for k in range(8)]
            Sm_t = [cpool.tile([128, H], F32R, tag=f"sm{k}", name=f"sm{k}") for k in range(8)]
            for k in range(8):
                nc.sync.dma_start(Cm_t[k][:], cm_p[128 * k : 128 * k + 128, :])
                nc.sync.dma_start(Sm_t[k][:], sm_p[128 * k : 128 * k + 128, :])
            bm_t = [
                cpool.tile([128, max(1, sum(hi - lo for (lo, hi, m) in table[ci]))],
                           BF16, tag=f"bm{ci}", name=f"bm{ci}")
                for ci in range(5)
            ]
            mf_t = [cpool.tile([128, NQ], BF16, tag=f"mf{ci}", name=f"mf{ci}") for ci in range(5)]
            sw_t = cpool.tile([128, 10], F32, tag="sw")
            nc.sync.dma_start(sw_t[:], sw_p[:])
            ic_t = cpool.tile([1, WB], F32, tag="ic")
            nc.sync.dma_start(ic_t[:], ic_p[:])
            ident = cpool.tile([128, 128], F32, tag="ident")
            make_identity(nc, ident[:])
            ones32 = cpool.tile([128, 1], F32, tag="ones32")
            nc.gpsimd.memset(ones32[:], 1.0)
            ones = cpool.tile([128, 1], F32R, tag="ones")
            nc.vector.tensor_copy(ones[:], ones32[:])
            zt = cpool.tile([128, WB], F32, tag="zt")
            nc.gpsimd.memset(zt[:], 0.0)
            lossv = cpool.tile([1, IMGS_PER_CORE], F32, tag="lossv")

            # per-image persistent arrays
            Xt = [xpool.tile([128, H], F32R, tag=f"x{k}", name=f"x{k}") for k in range(8)]
            Arn = [apool.tile([128, NQ], F32R, tag=f"arn{m}", name=f"arn{m}") for m in range(8)]
            Tt = [apool.tile([128, NQ], F32R, tag=f"t{m}", name=f"tt{m}") for m in range(8)]
            Gt = [qpool.tile([128, 516], F32, tag=f"g{ci}", name=f"g{ci}") for ci in range(5)]
            Xb = [qpool.tile([128, WB], F32R, tag=f"xb{ci}", name=f"xb{ci}") for ci in range(5)]
            p2acc = qpool.tile([128, 8], F32R, tag="p2acc")

            for img in range(IMGS_PER_CORE):
                for k in range(8):
                    nc.sync.dma_start(Xt[k][:], x_p[img, 128 * k : 128 * k + 128, :])
                if img == 0:
                    # shear masks aren't needed until ~150us in; keep them
                    # behind the input/DFT constants in the DMA queue
                    for ci in range(5):
                        wci = sum(hi - lo for (lo, hi, m) in table[ci])
                        if wci > 0:
                            nc.sync.dma_start(
                                bm_t[ci][:, 0:wci],
                                bm_p[128 * ci : 128 * ci + 128, 0:wci],
                            )
                        nc.sync.dma_start(mf_t[ci][:], mf_p[128 * ci : 128 * ci + 128, :])

                # ---------------- step 1: A = X^T (C - iS), store Arn=-Ar, T=-Ai
                sc1 = nc.named_scope(f"s1_{img}"); sc1.__enter__()
                for m in range(8):
                    pr_lo = ps.tile([128, 512], F32, tag="pa")
                    pr_hi = ps.tile([128, 8], F32, tag="pd")
                    pt_lo = ps.tile([128, 512], F32, tag="pb")
                    for k in range(8):
                        lhs = Xt[k][:, 128 * m : 128 * m + 128]
                        st, sp = (k == 0), (k == 7)
                        nc.tensor.matmul(pr_lo[:], lhs, Cm_t[k][:, 0:512], start=st, stop=sp)
                        nc.tensor.matmul(pr_hi[:], lhs, Cm_t[k][:, 512:520], start=st, stop=sp)
                        nc.tensor.matmul(pt_lo[:], lhs, Sm_t[k][:, 0:512], start=st, stop=sp)
                    nc.vector.tensor_scalar_mul(Arn[m][:, 0:512], pr_lo[:], -1.0)
                    nc.vector.tensor_scalar_mul(Arn[m][:, 512:513], pr_hi[:, 0:1], -1.0)
                    nc.vector.tensor_copy(Tt[m][:, 0:512], pt_lo[:])
                    nc.vector.tensor_copy(Tt[m][:, 512:513], zt[:, 0:1])

                sc1.__exit__(None, None, None)
                # ---------------- step 2 + power + fold
                sc2 = nc.named_scope(f"s2_{img}"); sc2.__enter__()
                nc.vector.tensor_copy(p2acc[:, 1:8], zt[:, 0:7])
                for mu in range(5):
                    M = 128 if mu < 4 else 1
                    u0 = 128 * mu
                    U = wpool.tile([128, H], F32, tag="U", bufs=1)
                    for h in range(2):
                        v0 = 512 * h
                        zr = ps.tile([128, 512], F32, tag="pa")
                        zia = ps.tile([128, 512], F32, tag="pb")
                        zib = ps.tile([128, 512], F32, tag="pc")
                        for k in range(8):
                            st, sp = (k == 0), (k == 7)
                            arn = Arn[k][:, u0 : u0 + M]
                            tt = Tt[k][:, u0 : u0 + M]
                            cmr = Cm_t[k][:, v0 : v0 + 512]
                            smr = Sm_t[k][:, v0 : v0 + 512]
                            if mu < 4:
                                nc.tensor.matmul(zr[0:M], arn, cmr, start=st, stop=False)
                                nc.tensor.matmul(zr[0:M], tt, smr, start=False, stop=sp, skip_group_check=True)
                                nc.tensor.matmul(zia[0:M], tt, cmr, start=st, stop=sp)
                            else:
                                # Nyquist row: Ai (=-T) column 512 is exactly 0,
                                # so Zr = Ar*C and Zi = -Ar*S; skip zero matmuls
                                nc.tensor.matmul(zr[0:M], arn, cmr, start=st, stop=sp)
                            nc.tensor.matmul(zib[0:M], arn, smr, start=st, stop=sp)
                        sc_ap = sw_t[0:M, 2 * mu : 2 * mu + 1]
                        nc.scalar.activation(
                            U[0:M, v0 : v0 + 512], zr[0:M],
                            mybir.ActivationFunctionType.Square, scale=sc_ap,
                        )
                        t2sb = wpool.tile([128, 512], F32, tag="t2sb", bufs=1)
                        if mu < 4:
                            ziasb = wpool.tile([128, 512], F32, tag="ziasb", bufs=1)
                            nc.scalar.activation(
                                ziasb[0:M], zia[0:M],
                                mybir.ActivationFunctionType.Copy, scale=sc_ap,
                            )
                            nc.vector.scalar_tensor_tensor(
                                t2sb[0:M], zib[0:M], sc_ap, ziasb[0:M],
                                op0=AT.mult, op1=AT.subtract,
                            )
                        else:
                            nc.vector.scalar_tensor_tensor(
                                t2sb[0:M], zib[0:M], sc_ap, zt[0:M, 0:512],
                                op0=AT.mult, op1=AT.subtract,
                            )
                        t2sq = wpool.tile([128, 512], F32, tag="t2sq", bufs=1)
                        nc.scalar.activation(
                            t2sq[0:M], t2sb[0:M], mybir.ActivationFunctionType.Square
                        )
                        nc.vector.tensor_tensor(
                            out=U[0:M, v0 : v0 + 512], in0=U[0:M, v0 : v0 + 512],
                            in1=t2sq[0:M], op=AT.add,
                        )
                    # row sums of U^2 -> p2acc (U = w*P, so multiply by 1/w)
                    rs = wpool.tile([128, 1], F32, tag="rs")
                    if mu == 0:
                        # zero the DC term: kills the catastrophic p2/q2
                        # cancellation; bin-0 loss term is eps^2 either way
                        nc.vector.tensor_copy(U[0:1, 0:1], zt[0:1, 0:1])
                    # column fold first (U then dead, squared in place)
                    if mu == 4:
                        nc.vector.tensor_copy(Gt[4][:], zt[:, 0:516])
                    nc.vector.tensor_tensor(
                        out=Gt[mu][0:M, 1:512], in0=U[0:M, 1:512],
                        in1=U[0:M, 1023:512:-1], op=AT.add,
                    )
                    nc.vector.tensor_copy(Gt[mu][0:M, 0:1], U[0:M, 0:1])
                    nc.vector.tensor_copy(Gt[mu][0:M, 512:513], U[0:M, 512:513])
                    nc.scalar.activation(
                        U[0:M], U[0:M], mybir.ActivationFunctionType.Square,
                        accum_out=rs[0:M],
                    )
                    nc.vector.scalar_tensor_tensor(
                        p2acc[0:M, 0:1], rs[0:M], sw_t[0:M, 2 * mu + 1 : 2 * mu + 2],
                        p2acc[0:M, 0:1], op0=AT.mult,
                        op1=(AT.bypass if mu == 0 else AT.add),
                    )

                sc2.__exit__(None, None, None)
                # ---------------- diagonal fold: F = G + G^T (upper triangle)
                sc3 = nc.named_scope(f"df_{img}"); sc3.__enter__()
                for ci in range(4):
                    for cj in range(ci, 4):
                        tp = ps.tile([128, 128], F32, tag="pd")
                        nc.tensor.transpose(
                            tp[:], Gt[cj][:, 128 * ci : 128 * ci + 128], ident[:]
                        )
                        nc.vector.tensor_tensor(
                            out=Gt[ci][:, 128 * cj : 128 * cj + 128],
                            in0=Gt[ci][:, 128 * cj : 128 * cj + 128],
                            in1=tp[:], op=AT.add,
                        )
                    tp4 = ps.tile([128, 128], F32, tag="pd")
                    nc.tensor.transpose(
                        tp4[:], Gt[4][:, 128 * ci : 128 * ci + 128], ident[:]
                    )
                    nc.vector.tensor_tensor(
                        out=Gt[ci][:, 512:513], in0=Gt[ci][:, 512:513],
                        in1=tp4[:, 0:1], op=AT.add,
                    )

                sc3.__exit__(None, None, None)
                # ---------------- Xb assembly + barrel shear
                sc4 = nc.named_scope(f"brl_{img}"); sc4.__enter__()
                for ci in range(5):
                    nc.vector.tensor_tensor(
                        out=Xb[ci][:, 0:NQ], in0=Gt[ci][:, 0:NQ],
                        in1=mf_t[ci][:], op=AT.mult,
                    )
                    nc.vector.tensor_copy(Xb[ci][:, NQ:WB], zt[:, NQ:WB])
                    off = 0
                    for t in range(NROUNDS):
                        lo, hi, m = table[ci][t]
                        wdt = hi - lo
                        if wdt <= 0:
                            continue
                        bit = 1 << t
                        tmp = wpool.tile([128, 640], F32R, tag="btmp", bufs=1)
                        nc.vector.tensor_tensor(
                            out=tmp[:, 0:wdt], in0=Xb[ci][:, lo:hi],
                            in1=bm_t[ci][:, off : off + wdt], op=AT.mult,
                        )
                        nc.vector.tensor_tensor(
                            out=Xb[ci][:, lo:hi], in0=Xb[ci][:, lo:hi],
                            in1=tmp[:, 0:wdt], op=AT.subtract,
                        )
                        nc.vector.tensor_tensor(
                            out=Xb[ci][:, lo + bit : hi + bit],
                            in0=Xb[ci][:, lo + bit : hi + bit],
                            in1=tmp[:, 0:wdt], op=AT.add,
                        )
                        off += wdt

                sc4.__exit__(None, None, None)
                # ---------------- S reduce + loss
                sc5 = nc.named_scope(f"red_{img}"); sc5.__enter__()
                ps_lo = ps.tile([1, 512], F32, tag="pa")
                ps_hi = ps.tile([1, 216], F32, tag="pc")
                for ci in range(5):
                    st, sp = (ci == 0), (ci == 4)
                    nc.tensor.matmul(ps_lo[:], ones[:], Xb[ci][:, 0:512], start=st, stop=sp)
                    nc.tensor.matmul(ps_hi[:], ones[:], Xb[ci][:, 512:WB], start=st, stop=sp)
                ssq = wpool.tile([1, WB], F32, tag="ssq", bufs=1)
                nc.scalar.activation(ssq[0:1, 0:512], ps_lo[:], mybir.ActivationFunctionType.Square)
                nc.scalar.activation(ssq[0:1, 512:WB], ps_hi[:], mybir.ActivationFunctionType.Square)
                nc.vector.tensor_tensor(out=ssq[:], in0=ssq[:], in1=ic_t[:], op=AT.mult)
                q2 = wpool.tile([1, 1], F32, tag="q2")
                nc.vector.tensor_reduce(
                    q2[:], ssq[:], axis=mybir.AxisListType.X, op=AT.add
                )
                psp = ps.tile([1, 8], F32, tag="pd")
                nc.tensor.matmul(psp[:], ones[:], p2acc[:], start=True, stop=True)
                nc.vector.tensor_tensor(
                    out=lossv[0:1, img : img + 1], in0=psp[0:1, 0:1], in1=q2[:],
                    op=AT.subtract,
                )

                sc5.__exit__(None, None, None)

            nc.sync.dma_start(out_p[:], lossv[:])

    nc.compile()
    return nc


def _get_nc():
    if "nc" not in _CACHE:
        _CACHE["nc"] = _build_nc()
    return _CACHE["nc"]


# ---------------------------------------------------------------- entry point
def kernel(prob_cg: np.ndarray) -> np.ndarray:
    hc = _host_constants()
    nc = _get_nc()
    x = np.ascontiguousarray(prob_cg[:, 0, :, :].astype(np.float32))
    in_maps = []
    for i in range(N_CORES):
        in_maps.append(
            dict(
                x=x[2 * i : 2 * i + 2],
                cm=hc["Cm"], sm=hc["Sm"], bm=hc["bmask_bf16"], mf=hc["mfull_bf16"],
                sw=hc["swc"], ic=hc["invc"],
            )
        )
    trace = os.environ.get("AT_TRACE", "0") == "1"
    res = run_bass_kernel_spmd(nc, in_maps, core_ids=list(range(N_CORES)), trace=trace)
    if trace and res.exec_time_ns is not None:
        print(f"HW exec time: {res.exec_time_ns} ns")
        if res.per_core_scope_times:
            for kname, v in sorted(res.per_core_scope_times.items()):
                print(f"  scope {kname}: {v}")
        try:
            insts = res.instructions_and_trace[0]
            busy = {}
            for i in insts:
                eng = str(getattr(i, "engine", "?"))
                busy[eng] = busy.get(eng, 0) + (getattr(i, "dur", 0) or 0)
            print("  engine busy ns:", {k: int(v) for k, v in sorted(busy.items())})
        except Exception as e:
            print("  (engine busy agg failed:", e, ")")
    losses = np.concatenate([r["out"].reshape(-1) for r in res.results])
    loss = losses.mean() + (H * H) * (EPS * EPS)
    return np.float32(WA * loss)



# revision 19
# speedup vs baseline: 1.6596x; 1.6596x over previous
"""AnisotropySuppressionLoss on 8 TRN2 NeuronCores (Bass/Tile).

Quadrant-folded real-input 2D DFT in bf16: fold the image along both axes
(even/odd under r -> 1024-r and c -> 1024-c) so each of the four fold
components needs only a ~513^3 matmul per DFT stage (3x fewer MACs than the
half-spectrum form; bf16 doubles the PE rate and quarters LDWEIGHTS).

With CE/SO/CE2/SO2 the four step-2 partial sums (cos/sin x even/odd folds):
  Fr(v) = CE+SO, Fr(1024-v) = CE-SO, Fi(v) = CE2-SO2, Fi(1024-v) = CE2+SO2.
With sqrt(2*w_u) baked into the trig matrices:
  SQ := CE^2+SO^2+CE2^2+SO2^2  equals the quadrant fold w_u*(P(v)+P(1024-v)),
  X  := CE*SO - CE2*SO2,  and  Plo^2+Phi^2 = SQ^2/2 + 2*X^2
so the radial-bin input G is just SQ (cols 0/512 halved) and the p2 term
accumulates from SQ and X row-reductions. Radial segment sums use the exact
barrel-shift shear; loss_img = sum_w P^2 - sum_k S_k^2/c_k (+ HW*eps^2, host).
Data-parallel: batch 16 -> 2 images on each of 8 cores; host averages.
"""

import os
import sys

sys.path.insert(0, "/opt/trn_rl_repo")

import numpy as np

import concourse.bass as bass
import concourse.tile as tile
from concourse import bacc, mybir
from concourse.bass_utils import run_bass_kernel_spmd
from concourse.masks import make_identity

F32 = mybir.dt.float32
F32R = mybir.dt.float32r
BF16 = mybir.dt.bfloat16

H = 1024
NQ = 513          # quadrant size (0..512 per axis)
NB = 725          # radial bins 0..724
WB = 728          # barrel buffer width
NROUNDS = 8
N_CORES = 8
IMGS_PER_CORE = 2
WA = 0.002
EPS = 1e-12
CHUNKS = [(0, 128), (128, 256), (256, 384), (384, 512), (512, 513)]
FAMS = ("ee", "eo", "oe", "oo")

_CACHE = {}


# ---------------------------------------------------------------- host consts
def _gen_barrel_masks():
    """Per (chunk, round): (lo, hi, move_mask[128, hi-lo]) in quadrant coords.
    Cells (a, b>=a) carry delta = bin - b; each round moves cells with bit t
    set right by 2^t. Merges are exact (same remaining delta); validated."""
    rem = -np.ones((NQ, WB), dtype=np.int64)
    for a in range(NQ):
        cols = np.arange(a, NQ)
        bins = np.floor(np.sqrt(a * a + cols.astype(np.float64) ** 2)).astype(np.int64)
        rem[a, cols] = bins - cols
    table = [[] for _ in CHUNKS]
    for t in range(NROUNDS):
        bit = 1 << t
        move = (rem >= 0) & ((rem & bit) != 0)
        for ci, (c0, c1) in enumerate(CHUNKS):
            mv = move[c0:c1]
            cols_any = np.nonzero(mv.any(axis=0))[0]
            if len(cols_any) == 0:
                table[ci].append((0, 0, None))
            else:
                lo, hi = int(cols_any[0]), int(cols_any[-1]) + 1
                m = np.zeros((128, hi - lo), dtype=np.float32)
                m[: c1 - c0] = mv[:, lo:hi]
                table[ci].append((lo, hi, m))
        new_rem = -np.ones_like(rem)
        stay = (rem >= 0) & ~move
        new_rem[stay] = rem[stay]
        sr, sc = np.nonzero(move)
        dc = sc + bit
        landing = rem[sr, sc] - bit
        cur = new_rem[sr, dc]
        assert ((cur == -1) | (cur == landing)).all()
        new_rem[sr, dc] = landing
        rem = new_rem
    assert (rem[rem >= 0] == 0).all()
    return table


def _host_constants():
    if "consts" in _CACHE:
        return _CACHE["consts"]
    import ml_dtypes

    cu = np.arange(NQ, dtype=np.float64)
    cc = np.arange(H, dtype=np.float64)
    wu = np.full(NQ, 2.0)
    wu[0] = 1.0
    wu[512] = 1.0
    sqw = np.sqrt(wu) / 32.0          # step-1 col scale: sqrt(w_u), half of 1/H
    s2c = np.sqrt(2.0) / 32.0         # step-2 scale (makes SQ == G directly)

    angf = 2.0 * np.pi * np.outer(cc, cu) / H
    Cw = np.cos(angf) * sqw[None, :]      # [1024, 513]: even part of xe auto
    Swn = -np.sin(angf) * sqw[None, :]    # [1024, 513]: odd part auto
    ang = 2.0 * np.pi * np.outer(cu, cu) / H
    C2 = np.cos(ang) * s2c
    C2[512] *= 0.5                    # A512 rows / xe col 512 are 2x true
    S2 = (np.sin(ang) * s2c)[:512]

    # p2 per-partition weights: cols 2mu = 0.5/w_u, 2mu+1 = 2/w_u
    wc = np.zeros((128, 10), dtype=np.float32)
    for mu in range(5):
        c0, c1 = CHUNKS[mu]
        n = c1 - c0
        wc[:n, 2 * mu] = (0.5 / wu[c0:c1]).astype(np.float32)
        wc[:n, 2 * mu + 1] = (2.0 / wu[c0:c1]).astype(np.float32)

    # radial bin counts exactly as reference._radial_bins (unshifted coords)
    y = np.minimum(np.arange(H), H - np.arange(H))
    yy, xx = np.meshgrid(y, y, indexing="ij")
    dist = np.sqrt((xx.astype(np.float64)) ** 2 + yy.astype(np.float64) ** 2)
    bins_full = np.clip(dist.astype(np.int32), 0, NB - 1)
    counts = np.bincount(bins_full.reshape(-1), minlength=NB).astype(np.float64)
    invc = np.zeros((1, WB), dtype=np.float32)
    invc[0, :NB] = (1.0 / counts).astype(np.float32)

    table = _gen_barrel_masks()
    widths = [[(hi - lo) for (lo, hi, m) in table[ci]] for ci in range(5)]
    chunk_w = [max(1, sum(ws)) for ws in widths]
    maxw = max(chunk_w)
    bmask = np.zeros((640, maxw), dtype=np.float32)
    for ci in range(5):
        off = 0
        for (lo, hi, m) in table[ci]:
            if m is None:
                continue
            bmask[128 * ci : 128 * ci + 128, off : off + hi - lo] = m
            off += hi - lo

    # full-width pre-masks: 0 below diag, 0.5 on diag (doubled by fold),
    # 1 above; chunk 4 keeps its single diagonal cell at weight 1.
    mfull = np.zeros((640, NQ), dtype=np.float32)
    for ci in range(4):
        a = 128 * ci + np.arange(128)
        cols = np.arange(NQ)
        blk = (cols[None, :] > a[:, None]).astype(np.float32)
        blk[np.arange(128), a] = 0.5
        mfull[128 * ci : 128 * ci + 128] = blk
    mfull[512, 512] = 1.0

    bf = ml_dtypes.bfloat16
    _CACHE["consts"] = dict(
        cw=Cw.astype(bf), swn=Swn.astype(bf), c2=C2.astype(bf), s2=S2.astype(bf),
        wc=wc, invc=invc,
        bmask_bf16=bmask.astype(bf), mfull_bf16=mfull.astype(bf),
        table=table, maxw=maxw, counts=counts,
    )
    return _CACHE["consts"]


# ---------------------------------------------------------------- device build
def _build_nc():
    hc = _host_constants()
    table, maxw = hc["table"], hc["maxw"]

    nc = bacc.Bacc("TRN2", target_bir_lowering=False, debug=False)
    x_p = nc.declare_dram_parameter("x", [IMGS_PER_CORE, H, H], F32, isOutput=False)
    cw_p = nc.declare_dram_parameter("cw", [H, NQ], BF16, isOutput=False)
    swn_p = nc.declare_dram_parameter("swn", [H, NQ], BF16, isOutput=False)
    c2_p = nc.declare_dram_parameter("c2", [NQ, NQ], BF16, isOutput=False)
    s2_p = nc.declare_dram_parameter("s2", [512, NQ], BF16, isOutput=False)
    bm_p = nc.declare_dram_parameter("bm", [640, maxw], BF16, isOutput=False)
    mf_p = nc.declare_dram_parameter("mf", [640, NQ], BF16, isOutput=False)
    wc_p = nc.declare_dram_parameter("wc", [128, 10], F32, isOutput=False)
    ic_p = nc.declare_dram_parameter("ic", [1, WB], F32, isOutput=False)
    out_p = nc.declare_dram_parameter("out", [1, IMGS_PER_CORE], F32, isOutput=True)

    AT = mybir.AluOpType
    AF = mybir.ActivationFunctionType

    with tile.TileContext(nc) as tc:
        with (
            tc.tile_pool(name="const", bufs=1) as cpool,
            tc.tile_pool(name="xin", bufs=1) as xpool,
            tc.tile_pool(name="fold", bufs=1) as fpool,
            tc.tile_pool(name="amat", bufs=1) as apool,
            tc.tile_pool(name="quad", bufs=1) as qpool,
            tc.tile_pool(name="work", bufs=2) as wpool,
            tc.tile_pool(name="ps", bufs=2, space="PSUM") as ps,
            tc.tile_pool(name="ps1", bufs=1, space="PSUM") as ps1,
        ):
            # ---------------- constants / input DMA
            Xt = [xpool.tile([128, H], F32, tag=f"x{k}", name=f"x{k}") for k in range(8)]
            for k in range(8):
                nc.sync.dma_start(Xt[k][:], x_p[0, 128 * k : 128 * k + 128, :])
            cw_t = [cpool.tile([128, NQ], BF16, tag=f"cw{k}") for k in range(4)]
            swn_t = [cpool.tile([128, NQ], BF16, tag=f"sw{k}") for k in range(4)]
            c2_t = [cpool.tile([128, NQ], BF16, tag=f"c2{k}") for k in range(4)]
            s2_t = [cpool.tile([128, NQ], BF16, tag=f"s2{k}") for k in range(4)]
            cwx_t = cpool.tile([1, NQ], BF16, tag="cwx")
            c2x_t = cpool.tile([1, NQ], BF16, tag="c2x")
            psh_t = cpool.tile([128, 128], BF16, tag="psh")
            nc.sync.dma_start(psh_t[:], psh_p[:])
            for k in range(4):
                nc.sync.dma_start(cw_t[k][:], cw_p[128 * k : 128 * k + 128, :])
                nc.sync.dma_start(swn_t[k][:], swn_p[128 * k : 128 * k + 128, :])
            nc.sync.dma_start(cwx_t[:], cw_p[512:513, :])
            for k in range(4):
                nc.sync.dma_start(c2_t[k][:], c2_p[128 * k : 128 * k + 128, :])
                nc.sync.dma_start(s2_t[k][:], s2_p[128 * k : 128 * k + 128, :])
            nc.sync.dma_start(c2x_t[:], c2_p[512:513, :])
            bm_t = [
                cpool.tile([128, max(1, sum(hi - lo for (lo, hi, m) in table[ci]))],
                           BF16, tag=f"bm{ci}", name=f"bm{ci}")
                for ci in range(5)
            ]
            mf_t = [cpool.tile([128, NQ], BF16, tag=f"mf{ci}") for ci in range(5)]
            wc_t = cpool.tile([128, 10], F32, tag="wc")
            nc.sync.dma_start(wc_t[:], wc_p[:])
            ic_t = cpool.tile([1, WB], F32, tag="ic")
            nc.sync.dma_start(ic_t[:], ic_p[:])
            ident = cpool.tile([128, 128], F32, tag="ident")
            make_identity(nc, ident[:])
            ones32 = cpool.tile([128, 1], F32, tag="ones32")
            nc.gpsimd.memset(ones32[:], 1.0)
            ones = cpool.tile([128, 1], F32R, tag="ones")
            nc.vector.tensor_copy(ones[:], ones32[:])
            one1 = cpool.tile([1, 1], BF16, tag="one1")
            nc.gpsimd.memset(one1[:], 1.0)
            zt = cpool.tile([128, WB], F32, tag="zt")
            nc.gpsimd.memset(zt[:], 0.0)
            lossv = cpool.tile([1, IMGS_PER_CORE], F32, tag="lossv")

            # ---------------- persistent per-image arrays (tags shared: WAR
            # deps order the two images)
            xe = [fpool.tile([128, NQ], BF16, tag=f"xe{k}") for k in range(8)]
            xo = [fpool.tile([128, NQ], BF16, tag=f"xo{k}") for k in range(8)]
            for k in range(8):
                nc.scalar.activation(xo[k][:, 0:1], zt[:, 0:1], mybir.ActivationFunctionType.Copy)  # col r=0 never written
            f_t = {fam: [fpool.tile([128, NQ], BF16, tag=f"f{fam}{b}") for b in range(4)]
                   for fam in FAMS}
            A_t = {fam: [apool.tile([128, 512], BF16, tag=f"A{fam}{m}") for m in range(4)]
                   for fam in FAMS}
            Acol = {fam: apool.tile([128, 4], BF16, tag=f"Ac{fam}") for fam in FAMS}
            A512 = {fam: apool.tile([1, NQ], BF16, tag=f"A5{fam}") for fam in ("ee", "oe")}
            Gt = [qpool.tile([128, 516], F32, tag=f"g{ci}", name=f"g{ci}") for ci in range(5)]
            Xb = [qpool.tile([128, WB], F32R, tag=f"xb{ci}", name=f"xb{ci}") for ci in range(5)]
            p2acc = qpool.tile([128, 1], F32R, tag="p2acc")

            # ---------------- emission helpers (phases, software-pipelined)
            def rfold(img):
                sc = nc.named_scope(f"rf_{img}"); sc.__enter__()
                for k in range(8):
                    nc.vector.tensor_tensor(
                        out=xe[k][:, 1:513], in0=Xt[k][:, 1:513],
                        in1=Xt[k][:, 1023:511:-1], op=AT.add,
                    )
                    nc.vector.tensor_tensor(
                        out=xo[k][:, 1:513], in0=Xt[k][:, 1:513],
                        in1=Xt[k][:, 1023:511:-1], op=AT.subtract,
                    )
                    nc.scalar.activation(xe[k][:, 0:1], Xt[k][:, 0:1], AF.Copy)
                sc.__exit__(None, None, None)

            def step1(img):
                # A components, full 1024 contraction over c (the even/odd
                # extraction under c -> 1024-c is automatic: cos rows are
                # c-symmetric, sin rows antisymmetric).
                #   Are = xe @ Cw, Aie = xe @ Swn, Aro = xo @ Cw, Aio = xo @ Swn
                sc = nc.named_scope(f"s1_{img}"); sc.__enter__()
                for fam, xsrc, rhsM in (
                    ("ee", xe, cw_t), ("oe", xe, swn_t),
                    ("eo", xo, cw_t), ("oo", xo, swn_t),
                ):
                    for m in range(4):
                        psA = ps.tile([128, 512], F32, tag="pbig", name="pbig")
                        psAc = ps.tile([128, 8], F32, tag="psml", name="psml")
                        for k in range(8):
                            st, sp = (k == 0), (k == 7)
                            lhs = xsrc[k][:, 128 * m : 128 * m + 128]
                            nc.tensor.matmul(psA[:], lhs, rhsM[k][:, 0:512],
                                             start=st, stop=sp)
                            nc.tensor.matmul(psAc[:, 0:1], lhs, rhsM[k][:, 512:513],
                                             start=st, stop=sp)
                        nc.scalar.activation(A_t[fam][m][:], psA[:], AF.Copy)
                        nc.scalar.activation(Acol[fam][:, m : m + 1], psAc[:, 0:1], AF.Copy)
                    if fam in ("ee", "oe"):
                        # r = 512 row of A (xe col 512 holds 2*x[:,512]; the
                        # 0.5 baked into C2 row 512 compensates)
                        psRf = ps1.tile([128, 512], F32, tag="p2ce", name="p2ce")
                        psR = psRf[0:1, :]
                        psRcf = ps.tile([128, 8], F32, tag="psml", name="psml")
                        psRc = psRcf[0:1, 0:1]
                        for k in range(8):
                            st, sp = (k == 0), (k == 7)
                            lhs = xe[k][:, 512:513]
                            nc.tensor.matmul(psR, lhs, rhsM[k][:, 0:512],
                                             start=st, stop=sp)
                            nc.tensor.matmul(psRc, lhs, rhsM[k][:, 512:513],
                                             start=st, stop=sp)
                        nc.scalar.activation(A512[fam][0:1, 0:512], psR, AF.Copy)
                        nc.scalar.activation(A512[fam][0:1, 512:513], psRc,
                                             AF.Copy)
                sc.__exit__(None, None, None)

            def step2(img):
                sc = nc.named_scope(f"s2_{img}"); sc.__enter__()
                for mu in range(5):
                    M = 128 if mu < 4 else 1
                    u0 = 128 * mu
                    psCE = ps1.tile([128, 512], F32, tag="p2ce")
                    psSO = ps1.tile([128, 512], F32, tag="p2so")
                    psCE2 = ps1.tile([128, 512], F32, tag="p2ce2")
                    psSO2 = ps1.tile([128, 512], F32, tag="p2so2")
                    psc = ps1.tile([128, 8], F32, tag="p2c")
                    for k in range(4):
                        st = (k == 0)
                        if mu < 4:
                            lee = A_t["ee"][k][:, u0 : u0 + M]
                            loe = A_t["oe"][k][:, u0 : u0 + M]
                            loo = A_t["oo"][k][:, u0 : u0 + M]
                            leo = A_t["eo"][k][:, u0 : u0 + M]
                        else:
                            lee = Acol["ee"][:, k : k + 1]
                            loe = Acol["oe"][:, k : k + 1]
                            loo = Acol["oo"][:, k : k + 1]
                            leo = Acol["eo"][:, k : k + 1]
                        nc.tensor.matmul(psCE[0:M], lee, c2_t[k][:, 0:512],
                                         start=st, stop=False)
                        nc.tensor.matmul(psc[0:M, 0:1], lee, c2_t[k][:, 512:513],
                                         start=st, stop=False)
                        nc.tensor.matmul(psCE2[0:M], loe, c2_t[k][:, 0:512],
                                         start=st, stop=False)
                        nc.tensor.matmul(psc2[0:M, 0:1], loe, c2_t[k][:, 512:513],
                                         start=st, stop=False)
                        nc.tensor.matmul(psSO[0:M], loo, s2_t[k][:, 0:512],
                                         start=st, stop=(k == 3))
                        nc.tensor.matmul(psSO2[0:M], leo, s2_t[k][:, 0:512],
                                         start=st, stop=(k == 3))
                    if mu < 4:
                        xee = A512["ee"][0:1, u0 : u0 + M]
                        xoe = A512["oe"][0:1, u0 : u0 + M]
                    else:
                        xee = A512["ee"][0:1, 512:513]
                        xoe = A512["oe"][0:1, 512:513]
                    nc.tensor.matmul(psCE[0:M], xee, c2x_t[0:1, 0:512],
                                     start=False, stop=True, skip_group_check=True)
                    nc.tensor.matmul(psc[0:M, 0:1], xee, c2x_t[0:1, 512:513],
                                     start=False, stop=True, skip_group_check=True)
                    nc.tensor.matmul(psCE2[0:M], xoe, c2x_t[0:1, 0:512],
                                     start=False, stop=True, skip_group_check=True)
                    nc.tensor.matmul(psc2[0:M, 0:1], xoe, c2x_t[0:1, 512:513],
                                     start=False, stop=True, skip_group_check=True)

                    # evacuate products to SBUF (bf16) on scalar, then all
                    # squares/products run on vector+gpsimd from SBUF at 2x
                    ces = wpool.tile([128, 512], BF16, tag="ces", name="ces")
                    sos = wpool.tile([128, 512], BF16, tag="sos", name="sos")
                    ce2s = wpool.tile([128, 512], BF16, tag="ce2s", name="ce2s")
                    so2s = wpool.tile([128, 512], BF16, tag="so2s", name="so2s")
                    sqe = wpool.tile([128, 2], F32, tag="sqe", name="sqe")
                    nc.scalar.activation(ces[0:M], psCE[0:M], AF.Copy)
                    nc.scalar.activation(sos[0:M], psSO[0:M], AF.Copy)
                    nc.scalar.activation(ce2s[0:M], psCE2[0:M], AF.Copy)
                    nc.scalar.activation(so2s[0:M], psSO2[0:M], AF.Copy)
                    # v=512 column carries the 0.5 edge factor via scale
                    nc.scalar.activation(sqe[0:M, 0:1], psc[0:M, 0:1], AF.Square,
                                         scale=float(np.sqrt(0.5)))
                    nc.scalar.activation(sqe[0:M, 1:2], psc2[0:M, 0:1], AF.Square,
                                         scale=float(np.sqrt(0.5)))
                    sqa = wpool.tile([128, 512], BF16, tag="sqa", name="sqa")
                    sqb = wpool.tile([128, 512], BF16, tag="sqb", name="sqb")
                    sqc = wpool.tile([128, 512], BF16, tag="sqc", name="sqc")
                    sqd = wpool.tile([128, 512], BF16, tag="sqd", name="sqd")
                    nc.vector.tensor_tensor(out=sqa[0:M], in0=ces[0:M], in1=ces[0:M],
                                            op=AT.mult)
                    nc.vector.tensor_tensor(out=sqb[0:M], in0=sos[0:M], in1=sos[0:M],
                                            op=AT.mult)
                    nc.vector.tensor_tensor(out=sqc[0:M], in0=ce2s[0:M], in1=ce2s[0:M],
                                            op=AT.mult)
                    nc.vector.tensor_tensor(out=sqd[0:M], in0=so2s[0:M], in1=so2s[0:M],
                                            op=AT.mult)
                    s1t = wpool.tile([128, 512], BF16, tag="s1t", name="s1t")
                    s2w = wpool.tile([128, 512], BF16, tag="s2w", name="s2w")
                    nc.vector.tensor_tensor(out=s1t[0:M], in0=sqa[0:M], in1=sqb[0:M],
                                            op=AT.add)
                    nc.vector.tensor_tensor(out=s2w[0:M], in0=sqc[0:M], in1=sqd[0:M],
                                            op=AT.add)
                    nc.vector.tensor_tensor(out=Gt[mu][0:M, 0:512], in0=s1t[0:M],
                                            in1=s2w[0:M], op=AT.add)
                    nc.vector.tensor_tensor(out=Gt[mu][0:M, 512:513], in0=sqe[0:M, 0:1],
                                            in1=sqe[0:M, 1:2], op=AT.add)
                    nc.vector.tensor_scalar_mul(Gt[mu][0:M, 0:1], Gt[mu][0:M, 0:1], 0.5)
                    if mu == 0:
                        nc.vector.tensor_scalar_mul(Gt[0][0:1, 0:1], Gt[0][0:1, 0:1], 0.0)
                    # X = CE*SO - CE2*SO2
                    q1 = wpool.tile([128, 512], BF16, tag="q1", name="q1")
                    q2w = wpool.tile([128, 512], BF16, tag="q2w", name="q2w")
                    xp = wpool.tile([128, 512], BF16, tag="xp", name="xp")
                    nc.vector.tensor_tensor(out=q1[0:M], in0=ces[0:M], in1=sos[0:M],
                                            op=AT.mult)
                    nc.vector.tensor_tensor(out=q2w[0:M], in0=ce2s[0:M], in1=so2s[0:M],
                                            op=AT.mult)
                    nc.vector.tensor_tensor(out=xp[0:M], in0=q1[0:M], in1=q2w[0:M],
                                            op=AT.subtract)
                    # p2 row accumulators: srs = sum SQ^2, xrs = sum X^2,
                    # crs/crs2 = SQ[:,0]^2, SQ[:,512]^2 (edge compensation)
                    junk = wpool.tile([128, 513], BF16, tag="junk", bufs=1, name="junk")
                    srs = wpool.tile([128, 1], F32, tag="srs", name="srs")
                    xrs = wpool.tile([128, 1], F32, tag="xrs", name="xrs")
                    crs = wpool.tile([128, 1], F32, tag="crs", name="crs")
                    crs2 = wpool.tile([128, 1], F32, tag="crs2", name="crs2")
                    nc.scalar.activation(junk[0:M, 0:513], Gt[mu][0:M, 0:513],
                                         AF.Square, accum_out=srs[0:M])
                    nc.scalar.activation(junk[0:M, 0:512], xp[0:M], AF.Square,
                                         accum_out=xrs[0:M])
                    nc.scalar.activation(junk[0:M, 0:1], Gt[mu][0:M, 0:1],
                                         AF.Square, accum_out=crs[0:M])
                    nc.scalar.activation(junk[0:M, 1:2], Gt[mu][0:M, 512:513],
                                         AF.Square, accum_out=crs2[0:M])
                    # p2acc += srs*(0.5/w) + xrs*(2/w) + (crs+crs2)*(0.5/w)
                    first = (mu == 0)
                    nc.vector.scalar_tensor_tensor(
                        p2acc[0:M, 0:1], srs[0:M], wc_t[0:M, 2 * mu : 2 * mu + 1],
                        p2acc[0:M, 0:1], op0=AT.mult,
                        op1=(AT.bypass if first else AT.add),
                    )
                    nc.vector.scalar_tensor_tensor(
                        p2acc[0:M, 0:1], xrs[0:M], wc_t[0:M, 2 * mu + 1 : 2 * mu + 2],
                        p2acc[0:M, 0:1], op0=AT.mult, op1=AT.add,
                    )
                    nc.vector.scalar_tensor_tensor(
                        p2acc[0:M, 0:1], crs[0:M], wc_t[0:M, 2 * mu : 2 * mu + 1],
                        p2acc[0:M, 0:1], op0=AT.mult, op1=AT.add,
                    )
                    nc.vector.scalar_tensor_tensor(
                        p2acc[0:M, 0:1], crs2[0:M], wc_t[0:M, 2 * mu : 2 * mu + 1],
                        p2acc[0:M, 0:1], op0=AT.mult, op1=AT.add,
                    )
                sc.__exit__(None, None, None)

            def dfold(img):
                sc = nc.named_scope(f"df_{img}"); sc.__enter__()
                for ci in range(4):
                    for cj in range(ci, 4):
                        tp = ps1.tile([128, 128], F32, tag="pd")
                        nc.tensor.transpose(
                            tp[:], Gt[cj][:, 128 * ci : 128 * ci + 128], ident[:]
                        )
                        nc.vector.tensor_tensor(
                            out=Gt[ci][:, 128 * cj : 128 * cj + 128],
                            in0=Gt[ci][:, 128 * cj : 128 * cj + 128],
                            in1=tp, op=AT.add,
                        )
                    tp4 = ps1.tile([128, 128], F32, tag="pd")
                    nc.tensor.transpose(
                        tp4[:], Gt[4][:, 128 * ci : 128 * ci + 128], ident[:]
                    )
                    nc.vector.tensor_tensor(
                        out=Gt[ci][:, 512:513], in0=Gt[ci][:, 512:513],
                        in1=tp4[:, 0:1], op=AT.add,
                    )
                sc.__exit__(None, None, None)

            def barrel(img):
                sc = nc.named_scope(f"brl_{img}"); sc.__enter__()
                brl_eng = {0: nc.vector, 1: nc.vector, 2: nc.vector,
                           3: nc.vector, 4: nc.vector}
                for ci in range(5):
                    eng = brl_eng[ci]
                    tname = "btmpg" if eng is nc.gpsimd else "btmpv"
                    eng.tensor_tensor(
                        out=Xb[ci][:, 0:NQ], in0=Gt[ci][:, 0:NQ],
                        in1=mf_t[ci][:], op=AT.mult,
                    )
                    eng.tensor_copy(Xb[ci][:, NQ:WB], zt[:, NQ:WB])
                    off = 0
                    for t in range(NROUNDS):
                        lo, hi, m = table[ci][t]
                        wdt = hi - lo
                        if wdt <= 0:
                            continue
                        bit = 1 << t
                        tmp = wpool.tile([128, 640], F32R, tag=tname, bufs=1,
                                         name=tname)
                        eng.tensor_tensor(
                            out=tmp[:, 0:wdt], in0=Xb[ci][:, lo:hi],
                            in1=bm_t[ci][:, off : off + wdt], op=AT.mult,
                        )
                        eng.tensor_tensor(
                            out=Xb[ci][:, lo:hi], in0=Xb[ci][:, lo:hi],
                            in1=tmp[:, 0:wdt], op=AT.subtract,
                        )
                        eng.tensor_tensor(
                            out=Xb[ci][:, lo + bit : hi + bit],
                            in0=Xb[ci][:, lo + bit : hi + bit],
                            in1=tmp[:, 0:wdt], op=AT.add,
                        )
                        off += wdt
                sc.__exit__(None, None, None)

            def reduce(img):
                sc = nc.named_scope(f"red_{img}"); sc.__enter__()
                ps_lo = ps1.tile([1, 512], F32, tag="ps1r")
                ps_hi = ps1.tile([1, 216], F32, tag="psrh")
                for ci in range(5):
                    st, sp = (ci == 0), (ci == 4)
                    nc.tensor.matmul(ps_lo, ones[:], Xb[ci][:, 0:512], start=st, stop=sp)
                    nc.tensor.matmul(ps_hi, ones[:], Xb[ci][:, 512:WB], start=st, stop=sp)
                ssq = wpool.tile([1, WB], F32, tag="ssq", bufs=1)
                nc.scalar.activation(ssq[0:1, 0:512], ps_lo, AF.Square)
                nc.scalar.activation(ssq[0:1, 512:WB], ps_hi, AF.Square)
                nc.vector.tensor_tensor(out=ssq[:], in0=ssq[:], in1=ic_t[:], op=AT.mult)
                q2v = wpool.tile([1, 1], F32, tag="q2v")
                nc.vector.tensor_reduce(
                    q2v[:], ssq[:], axis=mybir.AxisListType.X, op=AT.add
                )
                psp = ps.tile([128, 8], F32, tag="psml")
                nc.tensor.matmul(psp[0:1, 0:8], ones[:], p2acc[:], start=True, stop=True)
                nc.vector.tensor_tensor(
                    out=lossv[0:1, img : img + 1], in0=psp[0:1, 0:1], in1=q2v[:],
                    op=AT.subtract,
                )
                sc.__exit__(None, None, None)

            # ---------------- software-pipelined schedule: img1 folds and
            # matmuls overlap img0's barrel/reduce tail
            rfold(0)
            for k in range(8):
                nc.sync.dma_start(Xt[k][:], x_p[1, 128 * k : 128 * k + 128, :])
            for ci in range(5):
                wci = sum(hi - lo for (lo, hi, m) in table[ci])
                if wci > 0:
                    nc.sync.dma_start(bm_t[ci][:, 0:wci],
                                      bm_p[128 * ci : 128 * ci + 128, 0:wci])
                nc.sync.dma_start(mf_t[ci][:], mf_p[128 * ci : 128 * ci + 128, :])
            step1(0)
            step2(0)
            rfold(1)
            dfold(0)
            barrel(0)
            step1(1)
            reduce(0)
            step2(1)
            dfold(1)
            barrel(1)
            reduce(1)

            nc.sync.dma_start(out_p[:], lossv[:])

    nc.compile()
    return nc


def _get_nc():
    if "nc" not in _CACHE:
        _CACHE["nc"] = _build_nc()
    return _CACHE["nc"]


# ---------------------------------------------------------------- entry point
def kernel(prob_cg: np.ndarray) -> np.ndarray:
    hc = _host_constants()
    nc = _get_nc()
    x = np.ascontiguousarray(prob_cg[:, 0, :, :].astype(np.float32))
    in_maps = []
    for i in range(N_CORES):
        in_maps.append(
            dict(
                x=x[2 * i : 2 * i + 2],
                cw=hc["cw"], swn=hc["swn"], c2=hc["c2"], s2=hc["s2"],
                bm=hc["bmask_bf16"], mf=hc["mfull_bf16"],
                wc=hc["wc"], ic=hc["invc"],
            )
        )
    trace = os.environ.get("AT_TRACE", "0") == "1"
    res = run_bass_kernel_spmd(nc, in_maps, core_ids=list(range(N_CORES)), trace=trace)
    if trace and res.exec_time_ns is not None:
        print(f"HW exec time: {res.exec_time_ns} ns")
        if res.profile_json:
            print(f"  profile json: {res.profile_json}")
        if res.per_core_scope_times:
            for kname, v in sorted(res.per_core_scope_times.items()):
                print(f"  scope {kname}: {v}")
    losses = np.concatenate([r["out"].reshape(-1) for r in res.results])
    loss = losses.mean() + (H * H) * (EPS * EPS)
    return np.float32(WA * loss)
